# revision 14
# baseline (speedup 1.0000x reference)
"""LSG (local-sparse-global) block attention on 8 trn2 NeuronCores.

Shape/config facts hardcoded from the problem spec:
  n=2 h=12 t=4096 d=64, BLOCK=128, SPARSE_BLOCK=128, SF=4 -> ts=1024, g=64.
Per 128-token query block b the key set is:
  local [128(b-1), 128(b+2)) | global (64, padded to 128) | sparse W1
  [32b-160,32b-32) | sparse W2 [32b+64,32b+192)           -> 704 real keys.

Sharding: n*h = 24 pairs, 3 per core (data parallel, no cross-core comm).

Device-side design (per pair, 32 blocks, processed 2 blocks/iter):
 - Host ships Q^T/K^T d-major, 64 rows duplicated to 128 partitions so
   consecutive K=64 matmuls run concurrently in the two PE row-groups.
   Q is prescaled by log2e/64 so PSUM scores are u = s*log2e/8 (s = true
   scaled attention score).
 - Masking is folded into the value side: softmax(s+m) = exp(s)*exp(m)
   normalized, so the host scales each key's value row and the appended
   ones-column by exp(mask) (zero at structural pads).
 - The exp is SPLIT across engines to break the ACT throughput wall:
     * ACT does exact exp on the local+global segments (512 of 768 key
       columns): exp(u * 8ln2).
     * DVE does the sparse segments (256 cols) via a custom 8-slice DVE
       op: monic cubic in u then 3 squarings = g^8 * e^s with ~2e-3
       relative error (softmax global scale cancels; the residual g^8 is
       folded into the sparse value rows host-side as g^-8).
 - ctx matmuls: stationary = probsT chunk (128x128 bf16, FWL); moving =
   (values | exp(mask)) chunk (128, 65). One PSUM group per block:
   cols 0:64 = unnormalized ctx, col 64 = softmax denominator.
 - Normalization happens on the HOST: the device only copies the raw
   (ctx|den) PSUM tile into a per-pair SBUF staging buffer (DVE), which
   is shipped once per pair with a single large DMA (gpsimd SWDGE).
"""

import numpy as np
import ml_dtypes
from contextlib import ExitStack

import concourse.bacc as bacc
import concourse.bass as bass
import concourse.tile as tile
from concourse import mybir
from concourse.bass_utils import run_bass_kernel_spmd
from concourse.tile import add_dep_helper

N, H, T, D = 2, 12, 4096, 64
TS, G = 1024, 64
NCORES = 8
PAIRS = (N * H) // NCORES  # 3
NBLK = T // 128            # 32
BF16 = ml_dtypes.bfloat16

LOG2E = float(np.log2(np.e))
EXP_SCALE = float(8.0 * np.log(2.0))   # ACT: e^s = exp(u * 8ln2)
# monic cubic p(u) = u^3 + CB u^2 + CC u + CD ~ CG * 2^u on [-1.05, 1.05]
# (weighted minimax, weight 2^(6u)); device computes ((p^2)^2)^2 = CG^8 e^s.
CB = 2.761031543753213
CC = 8.536082335991596
CD = 12.329324786612675
CG = 12.312953390542324
GINV8 = float(1.0 / np.float64(CG) ** 8)

LAST_RESULTS = None  # BassKernelResults of the most recent run (for test.py)


def _register_exp8():
    """Register the custom DVE op (monic cubic + 3 squarings) with the
    concourse custom-DVE table machinery. Idempotent."""
    import concourse.dve_ops as dvo
    from concourse.dve_spec import Spec, Src0, C0, C1, C2, lower, sq, _has_src1
    from concourse.dve_uop import DveOpSpec

    name = "EXP8_MONIC_CUBIC_ANT"
    if name in dvo._SUB_OPCODE_FOR_NAME:
        return next(op for op in dvo.OPS if op.name == name)
    p = ((Src0 + C0) * Src0 + C1) * Src0 + C2
    body = sq(sq(sq(p)))

    def ref(in0, in1, s0, s1, imm2):
        x = in0.astype(np.float32)
        q = ((x + s0) * x + s1) * x + imm2
        return ((q * q) ** 2) ** 2

    spec = Spec(body=body, reference=ref)
    row = dvo._CUSTOM_DVE_ROW_BASE + len(dvo.OPS)
    dvo._SUB_OPCODE_FOR_NAME[name] = row
    shas = {}
    for ver in ("v3", "v4"):
        uops = lower(spec, ver=ver)
        s = DveOpSpec(name=name, opcode=row, uops=uops, rd1_en=_has_src1(spec))
        shas[ver] = s.sha(ver)
    op = dvo.DveOp(name, spec, subdim=False, uops_sha=shas)
    dvo.OPS.append(op)
    dvo.CUSTOM_DVE_SPECS[name] = spec
    return op


EXP8 = _register_exp8()


def build_program(pairs=PAIRS):
    dt = mybir.dt
    nc = bacc.Bacc("TRN2", target_bir_lowering=False, debug=False)

    qtb = nc.dram_tensor("qtb", [pairs, 128, T], dt.bfloat16, kind="ExternalInput").ap()
    ktb = nc.dram_tensor("ktb", [pairs, 128, T + 256], dt.bfloat16, kind="ExternalInput").ap()
    stb = nc.dram_tensor("stb", [pairs, 128, TS + 320], dt.bfloat16, kind="ExternalInput").ap()
    gtb = nc.dram_tensor("gtb", [pairs, 128, 128], dt.bfloat16, kind="ExternalInput").ap()
    vlb = nc.dram_tensor("vlb", [pairs, 128, 34, D + 1], dt.bfloat16, kind="ExternalInput").ap()
    svb = nc.dram_tensor("svb", [pairs, 128, 4, 10, D + 1], dt.bfloat16, kind="ExternalInput").ap()
    gvb = nc.dram_tensor("gvb", [pairs, 128, D + 1], dt.bfloat16, kind="ExternalInput").ap()
    # raw (ctx | denominator) per query: host divides.
    oT = nc.dram_tensor("oT", [pairs, 128, NBLK // 2, 2, D + 1], dt.float32,
                        kind="ExternalOutput").ap()

    # Scores go to TWO PSUM tiles so the two exp engines never touch the
    # same tile (the tile framework serializes same-tile access across
    # engines): ps_act [128,2,512] (banks 0-1: b0 in bank0, b1 in bank1)
    # holds local+global segments; ps_dve [128,2,256] (one bank) holds the
    # sparse segments. Concurrent row-group matmul pairs are chosen
    # bipartite (b0-local with b1-sparse, etc.) so each pair still lands
    # in two DIFFERENT PSUM banks - same-bank concurrent pairs are fatal.

    with tile.TileContext(nc) as tc, ExitStack() as ctx:
        big = ctx.enter_context(tc.tile_pool(name="big", bufs=2))
        probs = ctx.enter_context(tc.tile_pool(name="probs", bufs=3))
        outp = ctx.enter_context(tc.tile_pool(name="outp", bufs=2))
        ps_pool = ctx.enter_context(tc.tile_pool(name="psp", bufs=2, space="PSUM"))
        cx_pool = ctx.enter_context(tc.tile_pool(name="cxp", bufs=2, space="PSUM"))

        NITER = NBLK // 2
        for p in range(pairs):
            # Inputs arrive in two waves: first the slices iterations 0..3
            # need (so compute starts ~3us into the pair), then the rest.
            eng_a = nc.sync
            qt = big.tile([128, T], dt.bfloat16, tag="qt")
            kt = big.tile([128, T + 256], dt.bfloat16, tag="kt")
            st = big.tile([128, TS + 320], dt.bfloat16, tag="st")
            gt = big.tile([128, 128], dt.bfloat16, tag="gt")
            vl = big.tile([128, 34, D + 1], dt.bfloat16, tag="vl")
            sv = big.tile([128, 4, 10, D + 1], dt.bfloat16, tag="sv")
            gv = big.tile([128, D + 1], dt.bfloat16, tag="gv")
            QC, KC, SC, VC, JC = 1024, 1408, 640, 11, 4
            eng_a.dma_start(out=qt[:, 0:QC], in_=qtb[p][:, 0:QC])
            eng_a.dma_start(out=kt[:, 0:KC], in_=ktb[p][:, 0:KC])
            eng_a.dma_start(out=st[:, 0:SC], in_=stb[p][:, 0:SC])
            eng_a.dma_start(out=gt, in_=gtb[p])
            eng_a.dma_start(out=vl[:, 0:VC], in_=vlb[p][:, 0:VC])
            eng_a.dma_start(out=sv[:, :, 0:JC], in_=svb[p][:, :, 0:JC])
            eng_a.dma_start(out=gv, in_=gvb[p])
            eng_a.dma_start(out=qt[:, QC:], in_=qtb[p][:, QC:])
            eng_a.dma_start(out=kt[:, KC:], in_=ktb[p][:, KC:])
            eng_a.dma_start(out=st[:, SC:], in_=stb[p][:, SC:])
            eng_a.dma_start(out=vl[:, VC:], in_=vlb[p][:, VC:])
            eng_a.dma_start(out=sv[:, :, JC:], in_=svb[p][:, :, JC:])

            ob = outp.tile([128, NBLK // 2, 2, D + 1], dt.float32, tag="ob")

            # Software-pipelined: scores(i) are emitted one iteration ahead
            # of exp/ctx(i-1) so the tensor queue never round-trips through
            # the DVE exp before the next score group can issue.
            ps_tiles = {}
            cx_tiles = {}

            def emit_scores(i):
                blocks = (2 * i, 2 * i + 1)
                psa = ps_pool.tile([128, 2, 512], dt.float32, tag="psa")
                psv = ps_pool.tile([128, 2, 256], dt.float32, tag="psv")
                ps_tiles[i] = (psa, psv)
                stats = {}
                for bi, b in enumerate(blocks):
                    stat = [kt[:, (b + j) * 128:(b + j + 1) * 128] for j in range(3)]
                    stat.append(gt)
                    stat.append(st[:, b * 32:b * 32 + 128])
                    stat.append(st[:, b * 32 + 224:b * 32 + 352])
                    stats[bi] = stat
                # si 0..3 (loc x3, glob) -> psa; si 4..5 (sparse) -> psv.
                def out_ap(si, bi):
                    if si < 4:
                        return psa[:, bi, si * 128:(si + 1) * 128]
                    return psv[:, bi, (si - 4) * 128:(si - 3) * 128]
                # bipartite pair order: every consecutive (A,B) row-group
                # pair reads/writes two different PSUM banks.
                order = [(0, 0), (4, 1), (1, 0), (5, 1), (2, 0), (0, 1),
                         (3, 0), (1, 1), (4, 0), (2, 1), (5, 0), (3, 1)]
                prev = None
                for si, bi in order:
                    b = blocks[bi]
                    rows = slice(bi * 64, bi * 64 + 64)
                    inst = nc.tensor.matmul(
                        out_ap(si, bi),
                        stats[bi][si][rows, :],
                        qt[rows, b * 128:(b + 1) * 128],
                        start=True, stop=True)
                    if prev is not None:
                        add_dep_helper(inst.ins, prev.ins, sync=False)
                    prev = inst

            def emit_tail(i):
                blocks = (2 * i, 2 * i + 1)
                psa, psv = ps_tiles.pop(i)
                # --- probsT: ACT exact exp on local + b0-global; DVE approx
                # (custom op: monic cubic + 3 squarings = CG^8 e^s) on the
                # sparse segments and b1-global.
                pba = probs.tile([128, 2, 512], dt.bfloat16, tag="pba")
                pbv = probs.tile([128, 2, 256], dt.bfloat16, tag="pbv")
                nc.scalar.activation(pba, psa,
                                     mybir.ActivationFunctionType.Exp,
                                     scale=EXP_SCALE)
                nc.vector._custom_dve(EXP8, out=pbv,
                                      in0=psv,
                                      s0=CB, s1=CC, imm2=CD)
                # --- ctx + denominator, (q, d) layout. The ACT-dependent
                # matmuls are emitted first so they can run while the DVE
                # exp finishes.
                cx = cx_pool.tile([128, 2, D + 1], dt.float32, tag="cx")
                first_insts = {}
                last_inst = None
                for bi, b in enumerate(blocks):
                    ops = []
                    for j in range(3):
                        ops.append((pba[:, bi, j * 128:(j + 1) * 128], vl[:, b + j, :]))
                    ops.append((pba[:, bi, 384:512], gv))
                    ops.append((pbv[:, bi, 0:128], sv[:, b % 4, b // 4, :]))
                    w2 = b + 7
                    ops.append((pbv[:, bi, 128:256], sv[:, w2 % 4, w2 // 4, :]))
                    for oi, (lhsT, mov) in enumerate(ops):
                        start = (bi == 0 and oi == 0)
                        stop = (bi == 1 and oi == len(ops) - 1)
                        inst = nc.tensor.matmul(cx[:, bi, :],
                                                lhsT, mov,
                                                start=start, stop=stop)
                        if oi == 0:
                            first_insts[bi] = inst
                        last_inst_prev, last_inst = last_inst, inst
                add_dep_helper(first_insts[1].ins, first_insts[0].ins, sync=False)
                add_dep_helper(last_inst.ins, last_inst_prev.ins, sync=False)
                # --- stage raw (ctx | den); copy engine alternates per iter
                COPY_ALT = False
                if COPY_ALT and i % 2 == 1:
                    nc.scalar.copy(ob[:, i], cx)
                else:
                    nc.vector.tensor_copy(ob[:, i], cx)

            for i in range(NITER + 1):
                if i < NITER:
                    emit_scores(i)
                if i >= 1:
                    emit_tail(i - 1)
                if i - 1 == NITER // 2:
                    nc.gpsimd.dma_start(out=oT[p][:, 0:NITER // 2],
                                        in_=ob[:, 0:NITER // 2])
            nc.gpsimd.dma_start(out=oT[p][:, NITER // 2:],
                                in_=ob[:, NITER // 2:])

    nc.compile()
    return nc


def _prep_pair(q, k, v, am, sk, sv, sm, gk, gv, gm):
    """Build the device-layout arrays for one (n, h) pair. All inputs fp32
    numpy: q/k/v (T, D); am (T,); sk/sv (TS, D); sm (TS,); gk/gv (G, D);
    gm (G,). Returns dict of bf16 arrays."""
    def dup(x64):
        return np.concatenate([x64, x64], axis=0)

    def expm(mask_vals):
        # exp(mask): 1.0 for zero mask, 0.0 for -inf-ish masks
        with np.errstate(over="ignore", under="ignore"):
            return np.exp(np.minimum(mask_vals, 60.0)).astype(np.float32)

    qt = dup(q.T * np.float32(LOG2E / 64.0))

    kt = np.zeros((64, T + 256), np.float32)
    kt[:, 128:128 + T] = k.T
    kt = dup(kt)

    stm = np.zeros((64, TS + 320), np.float32)
    stm[:, 160:160 + TS] = sk.T
    stm = dup(stm)

    gt = np.zeros((64, 128), np.float32)
    gt[:, :G] = gk.T
    gt = dup(gt)

    # value side: row k scaled by exp(mask_k); appended col = exp(mask_k);
    # structural pads stay all-zero. Sparse rows additionally carry CG^-8
    # to cancel the DVE exp approximation's global scale.
    em = expm(am)
    vpad = np.zeros((T + 256, D + 1), np.float32)
    vpad[128:128 + T, :D] = v * em[:, None]
    vpad[128:128 + T, D] = em
    vlb = vpad.reshape(34, 128, D + 1).transpose(1, 0, 2)

    esm = expm(sm) * np.float32(GINV8)
    spad = np.zeros((TS + 320, D + 1), np.float32)
    spad[160:160 + TS, :D] = sv * esm[:, None]
    spad[160:160 + TS, D] = esm
    svb = np.zeros((128, 4, 10, D + 1), np.float32)
    for r in range(4):
        nj = 10 if r < 3 else 9
        for j in range(nj):
            svb[:, r, j] = spad[32 * r + 128 * j: 32 * r + 128 * j + 128]

    egm = expm(gm)
    gvb = np.zeros((128, D + 1), np.float32)
    gvb[:G, :D] = gv * egm[:, None]
    gvb[:G, D] = egm

    return dict(qtb=qt.astype(BF16), ktb=kt.astype(BF16), stb=stm.astype(BF16),
                gtb=gt.astype(BF16), vlb=vlb.astype(BF16), svb=svb.astype(BF16),
                gvb=gvb.astype(BF16))


def prep_inputs(inputs):
    """Full inputs -> list of per-core in_maps."""
    q = np.asarray(inputs["query_layer"], np.float32)
    k = np.asarray(inputs["key_layer"], np.float32)
    v = np.asarray(inputs["value_layer"], np.float32)
    am = np.asarray(inputs["attention_mask"], np.float32)[:, 0, 0, :]
    sk = np.asarray(inputs["sparse_key"], np.float32)
    sv = np.asarray(inputs["sparse_value"], np.float32)
    sm = np.asarray(inputs["sparse_mask"], np.float32)[:, 0, 0, :]
    gk = np.asarray(inputs["global_key"], np.float32)
    gv = np.asarray(inputs["global_value"], np.float32)
    gm = np.asarray(inputs["global_mask"], np.float32)[:, 0, 0, :]

    in_maps = []
    for c in range(NCORES):
        per_key = {}
        for pp in range(PAIRS):
            pair = c * PAIRS + pp
            n, h = divmod(pair, H)
            d = _prep_pair(q[n, h], k[n, h], v[n, h], am[n],
                           sk[n, h], sv[n, h], sm[n], gk[n, h], gv[n, h], gm[n])
            for name, arr in d.items():
                per_key.setdefault(name, []).append(arr)
        in_maps.append({name: np.stack(arrs) for name, arrs in per_key.items()})
    return in_maps


_prog_cache = {}


def _get_program():
    if "nc" not in _prog_cache:
        _prog_cache["nc"] = build_program()
    return _prog_cache["nc"]


def kernel(**inputs):
    global LAST_RESULTS
    nc = _get_program()
    in_maps = prep_inputs(inputs)
    res = run_bass_kernel_spmd(nc, in_maps, list(range(NCORES)))
    LAST_RESULTS = res
    out = np.empty((N, H, T, D), np.float32)
    for c in range(NCORES):
        oT = res.results[c]["oT"]  # (PAIRS, 128, 16, 2, 65) raw ctx|den
        for pp in range(PAIRS):
            pair = c * PAIRS + pp
            n, h = divmod(pair, H)
            raw = oT[pp]                      # (q=128, i=16, b=2, 65)
            ctx = raw[..., :D] / raw[..., D:D + 1]
            # token index = i*256 + b*128 + q
            out[n, h] = ctx.transpose(1, 2, 0, 3).reshape(T, D)
    return out


# revision 15
# speedup vs baseline: 1.1214x; 1.1214x over previous
"""LSG (local-sparse-global) block attention on 8 trn2 NeuronCores.

Shape/config facts hardcoded from the problem spec:
  n=2 h=12 t=4096 d=64, BLOCK=128, SPARSE_BLOCK=128, SF=4 -> ts=1024, g=64.
Per 128-token query block b the key set is:
  local [128(b-1), 128(b+2)) | global (64, padded to 128) | sparse W1
  [32b-160,32b-32) | sparse W2 [32b+64,32b+192)           -> 704 real keys.

Sharding: n*h = 24 pairs, 3 per core (data parallel, no cross-core comm).

Device-side design (per pair, 32 blocks, processed 2 blocks/iter):
 - Host ships Q^T/K^T d-major, 64 rows duplicated to 128 partitions so
   consecutive K=64 matmuls run concurrently in the two PE row-groups.
   Q is prescaled by log2e/64 so PSUM scores are u = s*log2e/8 (s = true
   scaled attention score).
 - Masking is folded into the value side: softmax(s+m) = exp(s)*exp(m)
   normalized, so the host scales each key's value row and the appended
   ones-column by exp(mask) (zero at structural pads).
 - The exp is SPLIT across engines to break the ACT throughput wall:
     * ACT does exact exp on the local+global segments (512 of 768 key
       columns): exp(u * 8ln2).
     * DVE does the sparse segments (256 cols) via a custom 8-slice DVE
       op: monic cubic in u then 3 squarings = g^8 * e^s with ~2e-3
       relative error (softmax global scale cancels; the residual g^8 is
       folded into the sparse value rows host-side as g^-8).
 - ctx matmuls: stationary = probsT chunk (128x128 bf16, FWL); moving =
   (values | exp(mask)) chunk (128, 65). One PSUM group per block:
   cols 0:64 = unnormalized ctx, col 64 = softmax denominator.
 - Normalization happens on the HOST: the device only copies the raw
   (ctx|den) PSUM tile into a per-pair SBUF staging buffer (DVE), which
   is shipped once per pair with a single large DMA (gpsimd SWDGE).
"""

import numpy as np
import ml_dtypes
from contextlib import ExitStack

import concourse.bacc as bacc
import concourse.bass as bass
import concourse.tile as tile
from concourse import mybir
from concourse.bass_utils import run_bass_kernel_spmd
from concourse.tile import add_dep_helper

N, H, T, D = 2, 12, 4096, 64
TS, G = 1024, 64
NCORES = 8
PAIRS = (N * H) // NCORES  # 3
NBLK = T // 128            # 32
BF16 = ml_dtypes.bfloat16

LOG2E = float(np.log2(np.e))
EXP_SCALE = float(8.0 * np.log(2.0))   # ACT: e^s = exp(u * 8ln2)
# monic cubic p(u) = u^3 + CB u^2 + CC u + CD ~ CG * 2^u on [-1.05, 1.05]
# (weighted minimax, weight 2^(6u)); device computes ((p^2)^2)^2 = CG^8 e^s.
CB = 2.761031543753213
CC = 8.536082335991596
CD = 12.329324786612675
CG = 12.312953390542324
GINV8 = float(1.0 / np.float64(CG) ** 8)

LAST_RESULTS = None  # BassKernelResults of the most recent run (for test.py)


def _register_exp8():
    """Register the custom DVE op (monic cubic + 3 squarings) with the
    concourse custom-DVE table machinery. Idempotent."""
    import concourse.dve_ops as dvo
    from concourse.dve_spec import Spec, Src0, C0, C1, C2, lower, sq, _has_src1
    from concourse.dve_uop import DveOpSpec

    name = "EXP8_MONIC_CUBIC_ANT"
    if name in dvo._SUB_OPCODE_FOR_NAME:
        return next(op for op in dvo.OPS if op.name == name)
    p = ((Src0 + C0) * Src0 + C1) * Src0 + C2
    body = sq(sq(sq(p)))

    def ref(in0, in1, s0, s1, imm2):
        x = in0.astype(np.float32)
        q = ((x + s0) * x + s1) * x + imm2
        return ((q * q) ** 2) ** 2

    spec = Spec(body=body, reference=ref)
    row = dvo._CUSTOM_DVE_ROW_BASE + len(dvo.OPS)
    dvo._SUB_OPCODE_FOR_NAME[name] = row
    shas = {}
    for ver in ("v3", "v4"):
        uops = lower(spec, ver=ver)
        s = DveOpSpec(name=name, opcode=row, uops=uops, rd1_en=_has_src1(spec))
        shas[ver] = s.sha(ver)
    op = dvo.DveOp(name, spec, subdim=False, uops_sha=shas)
    dvo.OPS.append(op)
    dvo.CUSTOM_DVE_SPECS[name] = spec
    return op


EXP8 = _register_exp8()


def build_program(pairs=PAIRS):
    dt = mybir.dt
    nc = bacc.Bacc("TRN2", target_bir_lowering=False, debug=False)

    qtb = nc.dram_tensor("qtb", [pairs, 128, T], dt.bfloat16, kind="ExternalInput").ap()
    ktb = nc.dram_tensor("ktb", [pairs, 128, T + 256], dt.bfloat16, kind="ExternalInput").ap()
    stb = nc.dram_tensor("stb", [pairs, 128, TS + 320], dt.bfloat16, kind="ExternalInput").ap()
    gtb = nc.dram_tensor("gtb", [pairs, 128, 128], dt.bfloat16, kind="ExternalInput").ap()
    vlb = nc.dram_tensor("vlb", [pairs, 128, 34, D + 1], dt.bfloat16, kind="ExternalInput").ap()
    svb = nc.dram_tensor("svb", [pairs, 128, 4, 10, D + 1], dt.bfloat16, kind="ExternalInput").ap()
    gvb = nc.dram_tensor("gvb", [pairs, 128, D + 1], dt.bfloat16, kind="ExternalInput").ap()
    # raw (ctx | denominator) per query: host divides.
    oT = nc.dram_tensor("oT", [pairs, 128, NBLK // 2, 2, D + 1], dt.float32,
                        kind="ExternalOutput").ap()

    # Scores go to TWO PSUM tiles so the two exp engines never touch the
    # same tile (the tile framework serializes same-tile access across
    # engines): ps_act [128,2,512] (banks 0-1: b0 in bank0, b1 in bank1)
    # holds local+global segments; ps_dve [128,2,256] (one bank) holds the
    # sparse segments. Concurrent row-group matmul pairs are chosen
    # bipartite (b0-local with b1-sparse, etc.) so each pair still lands
    # in two DIFFERENT PSUM banks - same-bank concurrent pairs are fatal.

    with tile.TileContext(nc) as tc, ExitStack() as ctx:
        big = ctx.enter_context(tc.tile_pool(name="big", bufs=2))
        probs = ctx.enter_context(tc.tile_pool(name="probs", bufs=3))
        outp = ctx.enter_context(tc.tile_pool(name="outp", bufs=2))
        ps_pool = ctx.enter_context(tc.tile_pool(name="psp", bufs=2, space="PSUM"))
        cx_pool = ctx.enter_context(tc.tile_pool(name="cxp", bufs=2, space="PSUM"))

        NITER = NBLK // 2
        for p in range(pairs):
            # Inputs arrive in two waves: first the slices iterations 0..3
            # need (so compute starts ~3us into the pair), then the rest.
            eng_a = nc.sync
            qt = big.tile([128, T], dt.bfloat16, tag="qt")
            kt = big.tile([128, T + 256], dt.bfloat16, tag="kt")
            st = big.tile([128, TS + 320], dt.bfloat16, tag="st")
            gt = big.tile([128, 128], dt.bfloat16, tag="gt")
            vl = big.tile([128, 34, D + 1], dt.bfloat16, tag="vl")
            sv = big.tile([128, 4, 10, D + 1], dt.bfloat16, tag="sv")
            gv = big.tile([128, D + 1], dt.bfloat16, tag="gv")
            eng_a.dma_start(out=qt, in_=qtb[p])
            eng_a.dma_start(out=kt, in_=ktb[p])
            eng_a.dma_start(out=st, in_=stb[p])
            eng_a.dma_start(out=gt, in_=gtb[p])
            eng_a.dma_start(out=vl, in_=vlb[p])
            eng_a.dma_start(out=sv, in_=svb[p])
            eng_a.dma_start(out=gv, in_=gvb[p])

            ob = outp.tile([128, NBLK // 2, 2, D + 1], dt.float32, tag="ob")

            # Software-pipelined: scores(i) are emitted one iteration ahead
            # of exp/ctx(i-1) so the tensor queue never round-trips through
            # the DVE exp before the next score group can issue.
            ps_tiles = {}
            cx_tiles = {}

            def emit_scores(i):
                blocks = (2 * i, 2 * i + 1)
                psa = ps_pool.tile([128, 2, 512], dt.float32, tag="psa")
                psv = ps_pool.tile([128, 2, 256], dt.float32, tag="psv")
                ps_tiles[i] = (psa, psv)
                stats = {}
                for bi, b in enumerate(blocks):
                    stat = [kt[:, (b + j) * 128:(b + j + 1) * 128] for j in range(3)]
                    stat.append(gt)
                    stat.append(st[:, b * 32:b * 32 + 128])
                    stat.append(st[:, b * 32 + 224:b * 32 + 352])
                    stats[bi] = stat
                # si 0..3 (loc x3, glob) -> psa; si 4..5 (sparse) -> psv.
                def out_ap(si, bi):
                    if si < 4:
                        return psa[:, bi, si * 128:(si + 1) * 128]
                    return psv[:, bi, (si - 4) * 128:(si - 3) * 128]
                # bipartite pair order: every consecutive (A,B) row-group
                # pair reads/writes two different PSUM banks.
                order = [(0, 0), (4, 1), (1, 0), (5, 1), (2, 0), (0, 1),
                         (3, 0), (1, 1), (4, 0), (2, 1), (5, 0), (3, 1)]
                prev = None
                for si, bi in order:
                    b = blocks[bi]
                    rows = slice(bi * 64, bi * 64 + 64)
                    inst = nc.tensor.matmul(
                        out_ap(si, bi),
                        stats[bi][si][rows, :],
                        qt[rows, b * 128:(b + 1) * 128],
                        start=True, stop=True)
                    if prev is not None:
                        add_dep_helper(inst.ins, prev.ins, sync=False)
                    prev = inst

            def emit_tail(i):
                blocks = (2 * i, 2 * i + 1)
                psa, psv = ps_tiles.pop(i)
                # --- probsT: ACT exact exp on local + b0-global; DVE approx
                # (custom op: monic cubic + 3 squarings = CG^8 e^s) on the
                # sparse segments and b1-global.
                pba = probs.tile([128, 2, 512], dt.bfloat16, tag="pba")
                pbv = probs.tile([128, 2, 256], dt.bfloat16, tag="pbv")
                nc.scalar.activation(pba, psa,
                                     mybir.ActivationFunctionType.Exp,
                                     scale=EXP_SCALE)
                nc.vector._custom_dve(EXP8, out=pbv,
                                      in0=psv,
                                      s0=CB, s1=CC, imm2=CD)
                # --- ctx + denominator, (q, d) layout. The ACT-dependent
                # matmuls are emitted first so they can run while the DVE
                # exp finishes.
                cx = cx_pool.tile([128, 2, D + 1], dt.float32, tag="cx")
                first_insts = {}
                last_inst = None
                for bi, b in enumerate(blocks):
                    ops = []
                    for j in range(3):
                        ops.append((pba[:, bi, j * 128:(j + 1) * 128], vl[:, b + j, :]))
                    ops.append((pba[:, bi, 384:512], gv))
                    ops.append((pbv[:, bi, 0:128], sv[:, b % 4, b // 4, :]))
                    w2 = b + 7
                    ops.append((pbv[:, bi, 128:256], sv[:, w2 % 4, w2 // 4, :]))
                    for oi, (lhsT, mov) in enumerate(ops):
                        start = (bi == 0 and oi == 0)
                        stop = (bi == 1 and oi == len(ops) - 1)
                        inst = nc.tensor.matmul(cx[:, bi, :],
                                                lhsT, mov,
                                                start=start, stop=stop)
                        if oi == 0:
                            first_insts[bi] = inst
                        last_inst_prev, last_inst = last_inst, inst
                add_dep_helper(first_insts[1].ins, first_insts[0].ins, sync=False)
                add_dep_helper(last_inst.ins, last_inst_prev.ins, sync=False)
                # --- stage raw (ctx | den); copy engine alternates per iter
                COPY_ALT = False
                if COPY_ALT and i % 2 == 1:
                    nc.scalar.copy(ob[:, i], cx)
                else:
                    nc.vector.tensor_copy(ob[:, i], cx)

            for i in range(NITER + 1):
                if i < NITER:
                    emit_scores(i)
                if i >= 1:
                    emit_tail(i - 1)
                if i - 1 == NITER // 2:
                    nc.gpsimd.dma_start(out=oT[p][:, 0:NITER // 2],
                                        in_=ob[:, 0:NITER // 2])
            nc.gpsimd.dma_start(out=oT[p][:, NITER // 2:],
                                in_=ob[:, NITER // 2:])

    nc.compile()
    return nc


def _prep_pair(q, k, v, am, sk, sv, sm, gk, gv, gm):
    """Build the device-layout arrays for one (n, h) pair. All inputs fp32
    numpy: q/k/v (T, D); am (T,); sk/sv (TS, D); sm (TS,); gk/gv (G, D);
    gm (G,). Returns dict of bf16 arrays."""
    def dup(x64):
        return np.concatenate([x64, x64], axis=0)

    def expm(mask_vals):
        # exp(mask): 1.0 for zero mask, 0.0 for -inf-ish masks
        with np.errstate(over="ignore", under="ignore"):
            return np.exp(np.minimum(mask_vals, 60.0)).astype(np.float32)

    qt = dup(q.T * np.float32(LOG2E / 64.0))

    kt = np.zeros((64, T + 256), np.float32)
    kt[:, 128:128 + T] = k.T
    kt = dup(kt)

    stm = np.zeros((64, TS + 320), np.float32)
    stm[:, 160:160 + TS] = sk.T
    stm = dup(stm)

    gt = np.zeros((64, 128), np.float32)
    gt[:, :G] = gk.T
    gt = dup(gt)

    # value side: row k scaled by exp(mask_k); appended col = exp(mask_k);
    # structural pads stay all-zero. Sparse rows additionally carry CG^-8
    # to cancel the DVE exp approximation's global scale.
    em = expm(am)
    vpad = np.zeros((T + 256, D + 1), np.float32)
    vpad[128:128 + T, :D] = v * em[:, None]
    vpad[128:128 + T, D] = em
    vlb = vpad.reshape(34, 128, D + 1).transpose(1, 0, 2)

    esm = expm(sm) * np.float32(GINV8)
    spad = np.zeros((TS + 320, D + 1), np.float32)
    spad[160:160 + TS, :D] = sv * esm[:, None]
    spad[160:160 + TS, D] = esm
    svb = np.zeros((128, 4, 10, D + 1), np.float32)
    for r in range(4):
        nj = 10 if r < 3 else 9
        for j in range(nj):
            svb[:, r, j] = spad[32 * r + 128 * j: 32 * r + 128 * j + 128]

    egm = expm(gm)
    gvb = np.zeros((128, D + 1), np.float32)
    gvb[:G, :D] = gv * egm[:, None]
    gvb[:G, D] = egm

    return dict(qtb=qt.astype(BF16), ktb=kt.astype(BF16), stb=stm.astype(BF16),
                gtb=gt.astype(BF16), vlb=vlb.astype(BF16), svb=svb.astype(BF16),
                gvb=gvb.astype(BF16))


def prep_inputs(inputs):
    """Full inputs -> list of per-core in_maps."""
    q = np.asarray(inputs["query_layer"], np.float32)
    k = np.asarray(inputs["key_layer"], np.float32)
    v = np.asarray(inputs["value_layer"], np.float32)
    am = np.asarray(inputs["attention_mask"], np.float32)[:, 0, 0, :]
    sk = np.asarray(inputs["sparse_key"], np.float32)
    sv = np.asarray(inputs["sparse_value"], np.float32)
    sm = np.asarray(inputs["sparse_mask"], np.float32)[:, 0, 0, :]
    gk = np.asarray(inputs["global_key"], np.float32)
    gv = np.asarray(inputs["global_value"], np.float32)
    gm = np.asarray(inputs["global_mask"], np.float32)[:, 0, 0, :]

    in_maps = []
    for c in range(NCORES):
        per_key = {}
        for pp in range(PAIRS):
            pair = c * PAIRS + pp
            n, h = divmod(pair, H)
            d = _prep_pair(q[n, h], k[n, h], v[n, h], am[n],
                           sk[n, h], sv[n, h], sm[n], gk[n, h], gv[n, h], gm[n])
            for name, arr in d.items():
                per_key.setdefault(name, []).append(arr)
        in_maps.append({name: np.stack(arrs) for name, arrs in per_key.items()})
    return in_maps


_prog_cache = {}


def _get_program():
    if "nc" not in _prog_cache:
        _prog_cache["nc"] = build_program()
    return _prog_cache["nc"]


def kernel(**inputs):
    global LAST_RESULTS
    nc = _get_program()
    in_maps = prep_inputs(inputs)
    res = run_bass_kernel_spmd(nc, in_maps, list(range(NCORES)))
    LAST_RESULTS = res
    out = np.empty((N, H, T, D), np.float32)
    for c in range(NCORES):
        oT = res.results[c]["oT"]  # (PAIRS, 128, 16, 2, 65) raw ctx|den
        for pp in range(PAIRS):
            pair = c * PAIRS + pp
            n, h = divmod(pair, H)
            raw = oT[pp]                      # (q=128, i=16, b=2, 65)
            ctx = raw[..., :D] / raw[..., D:D + 1]
            # token index = i*256 + b*128 + q
            out[n, h] = ctx.transpose(1, 2, 0, 3).reshape(T, D)
    return out
